# revision 26
# baseline (speedup 1.0000x reference)
"""Trainium2 Bass kernel for CausalSelfAttention with KV-prefix cache (v2).

Problem (hardcoded): B=2, T=2048, C=1024, H=16, D=64, P=2048.
Sharding: 8 cores = 2 (batch) x 4 (head groups of 4 heads).

v2 design vs v1 baseline (698us):
 - denominator rows come free from the AV matmuls via a ones column
   appended to V (M=65, per-head PSUM banks); the per-chunk [1,512]
   denominator matmuls and their LDWEIGHTS are gone (-110us PE busy).
 - scores PSUM is double-buffered so the PE streams scores for chunk
   g+1 while ScalarE exps chunk g (v1's single 6-bank group serialized
   PE against ScalarE for ~180us of idle).
 - everything is bf16 (x, W, q/k, cache_k, y, W_proj, output partials);
   fp8 was measured to blow the 2e-2 rel-err budget (2.4e-2 on CPU sim).
 - ScalarE runs Exp exclusively; every PSUM->SBUF copy is on DVE.
 - remaining QKV/V matmul blocks are woven into the attention loop with
   static deadlines so phase 1 hides in ScalarE's shadow.
 - y+denominator PSUM is copied to SBUF right after the last AV so the
   single-buffered y banks free immediately; reciprocal/broadcast/
   normalize run from SBUF off the critical path.
"""

import numpy as np
import ml_dtypes
from contextlib import ExitStack

import concourse.bacc as bacc
import concourse.tile as tile
import concourse.mybir as mybir
from concourse.bass_utils import run_bass_kernel_spmd

F32 = mybir.dt.float32
F32R = mybir.dt.float32r
BF16 = mybir.dt.bfloat16
EXP = mybir.ActivationFunctionType.Exp
MULT = mybir.AluOpType.mult

B, T, C, H, D, P = 2, 2048, 1024, 16, 64, 2048
HPC = 4            # heads per core
NPAIR = 2          # head pairs per core
TQ = 512           # query block (matmul moving dim)
KC = 128           # key chunk (PSUM partition dim)
SCALE = 1.0 / np.sqrt(D)

NT = T // TQ       # 4  query blocks
NPC = P // KC      # 16 prefix key chunks
NCK = C // 128     # 8  C contraction chunks
NTC = T // 128     # 16 current-key 128-chunks


def build_kernel(n_cores=8, dbg=False):
    nc = bacc.Bacc("TRN2", target_bir_lowering=False, debug=False,
                   num_devices=n_cores)
    dbg_t = {}
    if dbg:
        for nm, shp, dt_ in [("dbg_qT", [128, T], BF16), ("dbg_kT", [128, T], BF16),
                             ("dbg_vt", [128, NTC, HPC, 66], BF16),
                             ("dbg_eb", [128, 2, TQ], BF16),
                             ("dbg_yb", [65, TQ], F32),
                             ("dbg_ysb", [128, NT, TQ], BF16)]:
            dbg_t[nm] = nc.dram_tensor(nm, shp, dt_, kind="ExternalOutput").ap()

    xt = nc.dram_tensor("xt", [128, NCK, T], BF16, kind="ExternalInput").ap()
    wqk = nc.dram_tensor("wqk", [128, NCK, 4 * 128], BF16, kind="ExternalInput").ap()
    wv = nc.dram_tensor("wv", [128, NCK, HPC * D], BF16, kind="ExternalInput").ap()
    ckt = nc.dram_tensor("ckt", [NPAIR, 128, P], BF16, kind="ExternalInput").ap()
    cv = nc.dram_tensor("cv", [NPAIR, 128, 2, NPC, 66], BF16, kind="ExternalInput").ap()
    wp = nc.dram_tensor("wp", [NPAIR, 128, C], BF16, kind="ExternalInput").ap()
    masks = nc.dram_tensor("masks", [128, 4, TQ], BF16, kind="ExternalInput").ap()
    bsel = nc.dram_tensor("bsel", [65, 64], F32, kind="ExternalInput").ap()
    vones = nc.dram_tensor("vones", [128, NTC, HPC, 2], BF16, kind="ExternalInput").ap()
    zrd = nc.dram_tensor("zrd", [65, TQ], F32, kind="ExternalInput").ap()
    out_t = nc.dram_tensor("out_t", [C, T], BF16, kind="ExternalOutput").ap()

    with tile.TileContext(nc) as tc, ExitStack() as top:
        const = top.enter_context(tc.tile_pool(name="const", bufs=1))
        persist = top.enter_context(tc.tile_pool(name="persist", bufs=1))

        # ---- persistent SBUF ---------------------------------------------
        qT = [persist.tile([128, T], BF16, tag=f"qT{i}", name=f"qT{i}") for i in range(NPAIR)]
        kT = [persist.tile([128, T], BF16, tag=f"kT{i}", name=f"kT{i}") for i in range(NPAIR)]
        cktT = [persist.tile([128, P], BF16, tag=f"cktT{i}", name=f"cktT{i}") for i in range(NPAIR)]
        cvt = [persist.tile([128, 2, NPC, 66], BF16, tag=f"cvt{i}", name=f"cvt{i}") for i in range(NPAIR)]
        vt = persist.tile([128, NTC, HPC, 66], BF16, tag="vt", name="vt")
        wpt = [persist.tile([128, C], BF16, tag=f"wpt{i}", name=f"wpt{i}") for i in range(NPAIR)]
        ysb = [persist.tile([128, NT, TQ], BF16, tag=f"ysb{i}", name=f"ysb{i}") for i in range(NPAIR)]
        rdE = persist.tile([65, TQ], F32R, tag="rdE", name="rdE")
        rdO = persist.tile([65, TQ], F32R, tag="rdO", name="rdO")
        maskt = const.tile([128, 4, TQ], BF16, tag="maskt", name="maskt")
        bselt = const.tile([65, 64], F32R, tag="bselt", name="bselt")
        xtT = persist.tile([128, NCK, T], BF16, tag="xtT", name="xtT")
        wqkT = persist.tile([128, NCK, 4 * 128], BF16, tag="wqkT", name="wqkT")
        wvT = persist.tile([128, NCK, HPC * D], BF16, tag="wvT", name="wvT")

        # ---- input DMAs: the first 16 land on all 16 queues, so the
        # pieces the first matmuls/scores need go first.
        for kc_ in range(NCK):
            nc.sync.dma_start(xtT[:, kc_, 0:TQ], xt[:, kc_, 0:TQ])
        for kc_ in range(NCK):
            nc.sync.dma_start(wqkT[:, kc_, :], wqk[:, kc_, :])
        for j in range(4):
            nc.sync.dma_start(cktT[0][:, j * TQ:(j + 1) * TQ],
                              ckt[0, :, j * TQ:(j + 1) * TQ])
        for j in range(4):
            nc.sync.dma_start(cvt[0][:, :, 4 * j:4 * j + 4, :],
                              cv[0, :, :, 4 * j:4 * j + 4, :])
        for kc_ in range(NCK):
            nc.sync.dma_start(wvT[:, kc_, :], wv[:, kc_, :])
        nc.sync.dma_start(bselt[:], bsel[:, :].bitcast(F32R))
        for j in range(4):
            nc.sync.dma_start(maskt[:, j, :], masks[:, j, :])
        for j in range(4):
            nc.sync.dma_start(cktT[1][:, j * TQ:(j + 1) * TQ],
                              ckt[1, :, j * TQ:(j + 1) * TQ])
        for j in range(4):
            nc.sync.dma_start(cvt[1][:, :, 4 * j:4 * j + 4, :],
                              cv[1, :, :, 4 * j:4 * j + 4, :])
        for nb in range(1, NT):
            for kc_ in range(NCK):
                nc.sync.dma_start(xtT[:, kc_, nb * TQ:(nb + 1) * TQ],
                                  xt[:, kc_, nb * TQ:(nb + 1) * TQ])
        for pr in range(NPAIR):
            nc.sync.dma_start(wpt[pr][:], wp[pr, :, :])
        nc.sync.dma_start(vt[:, :, :, 64:66], vones[:, :, :, :])
        nc.sync.dma_start(rdE[:], zrd[:, :].bitcast(F32R))
        nc.sync.dma_start(rdO[:], zrd[:, :].bitcast(F32R))

        with ExitStack() as main:
            psA = main.enter_context(tc.tile_pool(name="psA", bufs=2, space="PSUM"))
            psY = main.enter_context(tc.tile_pool(name="psY", bufs=1, space="PSUM"))
            ps1 = main.enter_context(tc.tile_pool(name="ps1", bufs=2, space="PSUM"))
            ebp = main.enter_context(tc.tile_pool(name="ebp", bufs=3))
            ybf = main.enter_context(tc.tile_pool(name="ybf", bufs=4))
            ytmp = main.enter_context(tc.tile_pool(name="ytmp", bufs=2))
            stg = main.enter_context(tc.tile_pool(name="stg", bufs=4))

            # ---- phase-1 block thunks ------------------------------------
            # mc: 0 = q pair0, 1 = q pair1, 2 = k pair0, 3 = k pair1
            def qk_block_thunks(mc, nb):
                dest = (qT[0], qT[1], kT[0], kT[1])[mc]
                cell = {}
                thunks = []

                def mk_mm(kc_):
                    def f():
                        if "ps" not in cell:
                            cell["ps"] = ps1.tile([128, TQ], F32, tag="p1", name="p1")
                        nc.tensor.matmul(
                            cell["ps"][:],
                            wqkT[:, kc_, mc * 128:(mc + 1) * 128],
                            xtT[:, kc_, nb * TQ:(nb + 1) * TQ],
                            start=(kc_ == 0), stop=(kc_ == NCK - 1),
                            skip_group_check=True)
                    return f

                for kc_ in range(NCK):
                    thunks.append(mk_mm(kc_))

                def fin():
                    with nc.allow_low_precision(reason="q/k psum -> bf16 SBUF"):
                        nc.vector.tensor_copy(
                            dest[:, nb * TQ:(nb + 1) * TQ], cell["ps"][:])
                thunks.append(fin)
                return thunks

            def v_block_thunks(tc_):
                cell = {}
                thunks = []

                def mk_mm(kc_):
                    def f():
                        if "ps" not in cell:
                            cell["ps"] = ps1.tile([128, TQ], F32, tag="p1", name="p1")
                        nc.tensor.matmul(
                            cell["ps"][:, 0:HPC * D],
                            xtT[:, kc_, tc_ * 128:(tc_ + 1) * 128],
                            wvT[:, kc_, :],
                            start=(kc_ == 0), stop=(kc_ == NCK - 1),
                            skip_group_check=True)
                    return f

                for kc_ in range(NCK):
                    thunks.append(mk_mm(kc_))

                def fin():
                    with nc.allow_low_precision(reason="v psum -> bf16 SBUF"):
                        nc.vector.tensor_copy(
                            vt[:, tc_, :, 0:64], cell["ps"][:, 0:HPC * D])
                thunks.append(fin)
                return thunks

            # upfront: only the q block the very first scores need; everything
            # else weaves into the chunk loop.
            for th in qk_block_thunks(0, 0):
                th()

            # ---- weave schedule for the remaining blocks -----------------
            tb_start = {}
            gc = 0
            for tb in range(NT):
                for pr in range(NPAIR):
                    tb_start[(tb, pr)] = gc
                    gc += NPC + 4 * (tb + 1)
            total_chunks = gc  # 208

            # blocks emit atomically (all 9 instructions at one slot) so the
            # ps1 ring never holds a partially-emitted accumulation when a
            # later alloc (bcp/proj) wants the slot back.
            blocks = []
            blocks.append((NPC, qk_block_thunks(2, 0)))           # k p0 nb0
            blocks.append((tb_start[(0, 1)], qk_block_thunks(1, 0)))
            blocks.append((tb_start[(0, 1)] + NPC, qk_block_thunks(3, 0)))
            for tc_ in range(4):
                blocks.append((NPC + tc_, v_block_thunks(tc_)))
            for nb in range(1, NT):
                for mc in (0, 1):   # q blocks: needed at tb=nb start
                    blocks.append((tb_start[(nb, 0)], qk_block_thunks(mc, nb)))
                for mc in (2, 3):   # k block nb first used at current chunk 4*nb
                    blocks.append((tb_start[(nb, 0)] + NPC + 4 * nb,
                                   qk_block_thunks(mc, nb)))
            for tc_ in range(4, NTC):
                tb = tc_ // 4      # vt[tc] first used at current chunk tc of tb
                blocks.append((tb_start[(tb, 0)] + NPC + tc_, v_block_thunks(tc_)))
            blocks.sort(key=lambda x: x[0])

            schedule = {}
            prev = -3
            for dl, ths in blocks:
                slot = min(max(prev + 3, dl - 30), dl - 1)
                assert 0 <= slot < dl <= total_chunks, (slot, dl)
                prev = slot
                schedule.setdefault(slot, []).extend(ths)

            # ---- main attention loop -------------------------------------
            # Tail work that would stall the PE in program order (bcast
            # matmuls waiting on the 3.3us DVE reciprocals) is deferred
            # into the next pair's chunk stream.
            def proj_pieces(nb):
                out = []
                for j in range(4):
                    def piece(nb=nb, mcs=(2 * j, 2 * j + 1)):
                        for mc in mcs:
                            ps = ps1.tile([128, TQ], F32, tag="p1", name="pproj")
                            for pr2 in range(NPAIR):
                                nc.tensor.matmul(
                                    ps[:],
                                    wpt[pr2][:, mc * 128:(mc + 1) * 128],
                                    ysb[pr2][:, nb, :],
                                    start=(pr2 == 0), stop=(pr2 == NPAIR - 1),
                                    skip_group_check=True)
                            ot = stg.tile([128, TQ], BF16, tag="ot", name="ot")
                            with nc.allow_low_precision(reason="out psum -> bf16"):
                                nc.vector.tensor_copy(ot[:], ps[:])
                            nc.sync.dma_start(
                                out_t[mc * 128:(mc + 1) * 128,
                                      nb * TQ:(nb + 1) * TQ], ot[:])
                    out.append(piece)
                return out

            pending_tail = []
            pending_proj = []
            gc = 0
            for tb in range(NT):
                for pr in range(NPAIR):
                    nkc = NPC + 4 * (tb + 1)
                    ybe = psY.tile([65, TQ], F32, tag="yE", name="yE")
                    ybo = psY.tile([65, TQ], F32, tag="yO", name="yO")
                    for c in range(nkc):
                        for th in schedule.pop(gc, ()):
                            th()
                        gc += 1
                        if c == 11 and pending_tail:
                            tpr, ttb, fn = pending_tail.pop()
                            fn()
                            if tpr == 1:
                                pending_proj.extend(proj_pieces(ttb))
                        if c >= 15 and pending_proj:
                            pending_proj.pop(0)()
                        if c < NPC:
                            ksrc, klo = cktT[pr], c * KC
                        else:
                            ksrc, klo = kT[pr], (c - NPC) * KC
                        sb = psA.tile([128, 2, TQ], F32, tag="sb", name="sb")
                        for h in range(2):
                            nc.tensor.matmul(
                                sb[:, h, :],
                                ksrc[h * 64:(h + 1) * 64, klo:klo + KC],
                                qT[pr][h * 64:(h + 1) * 64, tb * TQ:(tb + 1) * TQ],
                                start=True, stop=True,
                                tile_position=(h * 64, 0),
                                skip_group_check=True)
                        eb = ebp.tile([128, 2, TQ], BF16, tag="eb", name="eb")
                        nc.scalar.activation(eb[:], sb[:], EXP, scale=SCALE)
                        if c >= NPC:
                            j = (c - NPC) - 4 * tb
                            if j >= 0:
                                for h in range(2):
                                    nc.vector.tensor_tensor(
                                        eb[:, h, :], eb[:, h, :],
                                        maskt[:, j, :], MULT)
                        if dbg and tb == 0 and pr == 0 and c == 0:
                            nc.sync.dma_start(dbg_t["dbg_eb"][:, :, :], eb[:])
                        if c < NPC:
                            vsrc_e = cvt[pr][:, 0, c, 0:65]
                            vsrc_o = cvt[pr][:, 1, c, 0:65]
                        else:
                            ck = c - NPC
                            vsrc_e = vt[:, ck, 2 * pr + 0, 0:65]
                            vsrc_o = vt[:, ck, 2 * pr + 1, 0:65]
                        st, sp = (c == 0), (c == nkc - 1)
                        nc.tensor.matmul(ybe[:], vsrc_e, eb[:, 0, :],
                                         start=st, stop=sp,
                                         tile_position=(0, 0),
                                         skip_group_check=True)
                        nc.tensor.matmul(ybo[:], vsrc_o, eb[:, 1, :],
                                         start=st, stop=sp,
                                         tile_position=(0, 0),
                                         skip_group_check=True)

                    # ---- tail: free y banks + start recips now; defer the
                    # PE bcast + normalize into the next pair's chunks.
                    ybfE = ybf.tile([65, TQ], F32, tag="ybf", name="ybfE")
                    ybfO = ybf.tile([65, TQ], F32, tag="ybf", name="ybfO")
                    nc.vector.tensor_copy(ybfE[:], ybe[:])
                    nc.vector.tensor_copy(ybfO[:], ybo[:])
                    with nc.allow_low_precision(reason="recip -> f32r for bcast mm"):
                        nc.vector.reciprocal(rdE[64:65, :], ybfE[64:65, :])
                        nc.vector.reciprocal(rdO[64:65, :], ybfO[64:65, :])
                    if dbg and tb == 0 and pr == 0:
                        nc.sync.dma_start(dbg_t["dbg_yb"][:, :], ybfE[:])

                    def tail(tb=tb, pr=pr, ybfE=ybfE, ybfO=ybfO):
                        bcpE = ps1.tile([128, TQ], F32, tag="p1", name="bcpE")
                        bcpO = ps1.tile([128, TQ], F32, tag="p1", name="bcpO")
                        nc.tensor.matmul(bcpE[0:64, :], bselt[:], rdE[:],
                                         start=True, stop=True,
                                         skip_group_check=True)
                        nc.tensor.matmul(bcpO[0:64, :], bselt[:], rdO[:],
                                         start=True, stop=True,
                                         skip_group_check=True)
                        with nc.allow_low_precision(reason="normalize -> bf16 y"):
                            nc.vector.tensor_tensor(
                                ysb[pr][0:64, tb, :], ybfE[0:64, :],
                                bcpE[0:64, :], MULT)
                            yo = ytmp.tile([64, TQ], BF16, tag="yo", name="yo")
                            nc.vector.tensor_tensor(
                                yo[:], ybfO[0:64, :], bcpO[0:64, :], MULT)
                        nc.sync.dma_start(ysb[pr][64:128, tb, :], yo[:])
                    pending_tail.append((pr, tb, tail))

            while pending_tail:
                tpr, ttb, fn = pending_tail.pop()
                fn()
                if tpr == 1:
                    pending_proj.extend(proj_pieces(ttb))
            while pending_proj:
                pending_proj.pop(0)()
            assert not schedule, f"unemitted extras: {sorted(schedule)}"
            if dbg:
                nc.sync.dma_start(dbg_t["dbg_qT"][:, :], qT[0][:])
                nc.sync.dma_start(dbg_t["dbg_kT"][:, :], kT[0][:])
                nc.sync.dma_start(dbg_t["dbg_vt"][:, :, :, :], vt[:])
                nc.sync.dma_start(dbg_t["dbg_ysb"][:, :, :], ysb[0][:])

    nc.compile()
    return nc


def make_in_maps(x, W_attn, W_proj, cache_k, cache_v, n_cores=8):
    """Shard full inputs into per-core input maps (host side)."""
    b_, t_, c_ = x.shape
    h_ = cache_k.shape[1]
    d_ = c_ // h_
    p_ = cache_k.shape[2]
    hpc = h_ // (n_cores // b_)
    Wq = W_attn[:, 0 * c_:1 * c_]
    Wk = W_attn[:, 1 * c_:2 * c_]
    Wv = W_attn[:, 2 * c_:3 * c_]
    mask_np = np.zeros((128, 4, TQ), np.float32)
    for j in range(4):
        mask_np[:, j, :] = (np.arange(TQ)[None, :] >=
                            (np.arange(128)[:, None] + j * 128)).astype(np.float32)
    bsel_np = np.zeros((65, 64), np.float32)
    bsel_np[64, :] = 1.0
    in_maps = []
    for core in range(n_cores):
        b = core // (n_cores // b_)
        h0 = (core % (n_cores // b_)) * hpc
        heads = list(range(h0, h0 + hpc))
        cols = np.concatenate([np.arange(h * d_, (h + 1) * d_) for h in heads])
        # x^T chunked: xt[p, kc, t] = x[b, t, kc*128+p]
        xt_np = np.ascontiguousarray(
            x[b].T.reshape(NCK, 128, t_).transpose(1, 0, 2))
        # W cols: [q pair0 | q pair1 | k pair0 | k pair1], each 128 wide
        wqk_cols = np.concatenate(
            [Wq[:, cols[0:128]], Wq[:, cols[128:256]],
             Wk[:, cols[0:128]], Wk[:, cols[128:256]]], axis=1)
        wqk_np = np.ascontiguousarray(
            wqk_cols.reshape(NCK, 128, 512).transpose(1, 0, 2))
        wv_np = np.ascontiguousarray(
            Wv[:, cols].reshape(NCK, 128, 256).transpose(1, 0, 2))
        npair = hpc // 2
        ckt_np = np.zeros((npair, 128, p_), np.float32)
        cv_np = np.zeros((npair, 128, 2, NPC, 66), np.float32)
        wp_np = np.zeros((npair, 128, c_), np.float32)
        for pr in range(npair):
            he, ho = heads[2 * pr], heads[2 * pr + 1]
            ckt_np[pr, 0:64] = cache_k[b, he].T
            ckt_np[pr, 64:128] = cache_k[b, ho].T
            for hh, hd in ((0, he), (1, ho)):
                cvr = cache_v[b, hd].reshape(NPC, KC, d_)   # [chunk, key, d]
                cv_np[pr, :, hh, :, 0:64] = cvr.transpose(1, 0, 2)
                cv_np[pr, :, hh, :, 64] = 1.0
            wp_np[pr, 0:64] = W_proj[he * d_:(he + 1) * d_]
            wp_np[pr, 64:128] = W_proj[ho * d_:(ho + 1) * d_]
        in_maps.append({
            "xt": xt_np.astype(ml_dtypes.bfloat16),
            "wqk": wqk_np.astype(ml_dtypes.bfloat16),
            "wv": wv_np.astype(ml_dtypes.bfloat16),
            "ckt": ckt_np.astype(ml_dtypes.bfloat16),
            "cv": cv_np.astype(ml_dtypes.bfloat16),
            "wp": wp_np.astype(ml_dtypes.bfloat16),
            "masks": mask_np.astype(ml_dtypes.bfloat16),
            "bsel": bsel_np,
            "vones": np.ones((128, NTC, HPC, 2), ml_dtypes.bfloat16),
            "zrd": np.zeros((65, TQ), np.float32),
        })
    return in_maps


def assemble_output(results, n_cores=8, b_=B, t_=T, c_=C):
    """Sum per-core partial out^T over head groups, transpose back."""
    out = np.zeros((b_, t_, c_), np.float32)
    per_b = n_cores // b_
    for b in range(b_):
        acc = np.zeros((c_, t_), np.float32)
        for i in range(per_b):
            acc += results[b * per_b + i]["out_t"].astype(np.float32)
        out[b] = acc.T
    return out


_NC_CACHE = {}


def kernel(x, W_attn, W_proj, cache_k, cache_v):
    x = np.asarray(x, np.float32)
    W_attn = np.asarray(W_attn, np.float32)
    W_proj = np.asarray(W_proj, np.float32)
    cache_k = np.asarray(cache_k, np.float32)
    cache_v = np.asarray(cache_v, np.float32)
    if "nc" not in _NC_CACHE:
        _NC_CACHE["nc"] = build_kernel()
    nc = _NC_CACHE["nc"]
    in_maps = make_in_maps(x, W_attn, W_proj, cache_k, cache_v)
    res = run_bass_kernel_spmd(nc, in_maps, list(range(8)))
    return assemble_output(res.results)
